# revision 44
# baseline (speedup 1.0000x reference)
"""Trainium2 Bass kernel for nn_Bottleneck_refine (grouped bottleneck + block mask).

Reference computation (per image b):
    m   = upsample(mask[b])            # [4,7,7] -> per-group 56x56 {0,1}
    t1  = conv1x1_g4(x * m1)           # 512 -> 128; 1x1 commutes with mask
    a1  = m . relu(s1*t1 + c1)
    t2  = conv3x3_g4(a1)               # 128 -> 128 (pad 1)
    a2  = m . relu(s2*t2 + c2)
    y   = relu(s3*conv1x1_g4(a2) + c3 + x)

Identity used: for m in {0,1}:  m*relu(z) == relu(m*z), and the 1x1 conv
commutes with per-pixel masking, so the input mask multiply is absorbed.

v4-v8 (from v3, 87.0us; fast-mode HW exec ~79-82us, the device itself is
bimodal with a ~+13us slow clock mode):
  - masks ship as tiny seeds (12.5KB + 100KB per image vs 1.6MB) and are
    applied via 0-stride broadcast APs on DVE/GPSIMD; kills 3.2MB/core of
    HBM mask traffic plus the big mask SBUF tiles.
  - a1h is no longer fully memset: only the 58x58 halo border is zeroed
    (4 small strided memsets), saving ~5.5us of gpsimd.
  - per-image prep + all load dma_starts live in a prologue; image 0
    arrives in chunk-pair-ordered quarters split across both HWDGE rings
    (the head is issue-bound at ~0.6us/dma_start), everything else on the
    sync ring only -- a dma_start on a compute engine's queue blocks all
    later compute on that engine when the ring backs up.
  - stage C is one scalar_tensor_tensor (psum + c3 + x) followed by one
    relu, emitted per 784-px half at every conv2 tap of the next
    superchunk; relus alternate ACT (g<2, early taps) / DVE (late taps);
    the last four units split PE-identity-residual (P) and DVE-STT (D)
    paths so the tail drains on two engines in parallel.
  - v8 scheduler-aware fixes: image-1 seed loads issue BEFORE its x
    loads (mrow[b1]'s DVE copy is scheduled early and head-of-line
    blocked the stage-B muls for ~10us waiting on its seed); conv1 cp3
    is deferred into k0's tap stream (k0's taps never read chunks 6-7,
    and the PE queue is emission-ordered); all stage-B mask-muls on
    gpsimd (it idles between stage-A muls, and DVE was the 2nd-busiest).
  - measured engine facts: MM queue is serial (conv2 33ns/MM is the
    4-cols/beat psum write floor; 128-part-out MMs are 1 col/beat);
    gpsimd tensor_scalar_max is ~30x slow, gp tensor_mul is fine; DVE
    tensor_tensor-class ops run 1x from PSUM, tensor_scalar 2x.

Sharding: data-parallel over batch, 2 images per core on 8 cores.
Per-core HBM traffic ~= 6.7 MB in + 6.4 MB out (bf16).

Layouts per image (all SBUF, [partition, free]):
  xb_g    [128, 1568] bf16 per (group, superchunk) (channel-major, row-major px)
  a1h     [128, 58*58] bf16 halo'd masked mid activation
  a2s     [128, 392] bf16 per (g, superchunk): partition 32j+co = chunk 4k+j
  mrow    [128, 56, 7] bf16: mask value per (pixel row, mask col) per group
  chunks: 7 image rows (392 px), 8 chunks, 2 superchunks of 4 chunks.

PE mapping:
  conv1: 128x32 column tiling, tile (0, 32g), psum banks pC0/pC1 alternating.
  conv2: 32x32 16-tile packing, tile (32g, 32j): row=group, col=chunk-in-sc.
         9 taps accumulate into bank pB[g]; output chunk-scrambled.
  conv3: 32x128 row tiling, tile (32j, 0); 2 chunks per 2-bank psum tile
         (pC0/pC1 per half h).
"""

import numpy as np

import concourse.bass as bass
import concourse.tile as tile
from concourse import bacc, mybir
from concourse.bass_utils import run_bass_kernel_spmd

F32 = mybir.dt.float32
BF16 = mybir.dt.bfloat16
EPS = 1e-5

N_CORES = 8
B_TOT = 16
B = B_TOT // N_CORES  # images per core
G = 4
CIN = 512
MID = 128
H = W = 56
PIX = H * W  # 3136
R = 7  # image rows per chunk
CH = R * W  # 392 pixels per chunk
NCH = H // R  # 8 chunks
SC = 4  # chunks per superchunk
NSC = NCH // SC  # 2 superchunks
HH = H + 2  # halo'd height/width (58)
SCW = SC * CH  # pixels per superchunk (1568)

# stage C path per unit in emission order (16 units = (b,k) x g):
#   "D": one fused DVE STT (psum + c3 + x) -> tmp, then relu on ACT (g<2)
#        or a non-in-place DVE max (in-place blocks the 4x bf16 mode).
#        Keep the STT fused: splitting it into tensor_scalar+add to get the
#        2x-PSUM mode regressed ~18us twice -- per-op DVE DRAIN flushes make
#        op COUNT the currency, exactly as the engine doc warns.
#        (STT must stay off gpsimd: no PSUM access; gp max is ~13.5us!)
#   "P": residual folded into PSUM via an identity matmul (PE has slack in
#        the back half), drained by a single fused ACT relu+bias from psum.
STC_PATH = ["D"] * 12 + ["P", "D", "P", "D"]


def build_nc():
    # Bacc (not Bass): its compile()/finalize() pipeline legalizes sync waits
    # (>=2 waits per instruction are split into EventSemaphore instructions,
    # which this walrus build requires) and moves matmul waits to ldweights.
    nc = bacc.Bacc(None, target_bir_lowering=False)

    xs = nc.dram_tensor("xs", [B, CIN, PIX], BF16, kind="ExternalInput")
    smd = nc.dram_tensor("smd", [B, 128, 49], BF16, kind="ExternalInput")
    ssd = nc.dram_tensor("ssd", [B, 128, G * NSC * 49], BF16, kind="ExternalInput")
    w1l = nc.dram_tensor("w1l", [128, G, 32], BF16, kind="ExternalInput")
    w2l = nc.dram_tensor("w2l", [128, 9, 32], BF16, kind="ExternalInput")
    w3l = nc.dram_tensor("w3l", [128, G, 128], BF16, kind="ExternalInput")
    b1d = nc.dram_tensor("b1d", [128, 1], F32, kind="ExternalInput")
    b2d = nc.dram_tensor("b2d", [128, G], F32, kind="ExternalInput")
    b3d = nc.dram_tensor("b3d", [128, G], F32, kind="ExternalInput")
    idm = nc.dram_tensor("idm", [128, 128], BF16, kind="ExternalInput")
    ys = nc.dram_tensor("ys", [B, CIN, PIX], BF16, kind="ExternalOutput")

    with tile.TileContext(nc) as tc:
        with (
            tc.tile_pool(name="consts", bufs=1) as consts,
            tc.tile_pool(name="xpool", bufs=16) as xpool,
            tc.tile_pool(name="mpool", bufs=2) as mpool,
            tc.tile_pool(name="a1pool", bufs=2) as a1pool,
            tc.tile_pool(name="a2pool", bufs=12) as a2pool,
            tc.tile_pool(name="upool", bufs=14) as upool,
            tc.tile_pool(name="opool", bufs=12) as opool,
            tc.tile_pool(name="psum", bufs=1, space="PSUM") as psum,
        ):
            # ---- constants (loaded once) ----
            w1sb = consts.tile([128, G, 32], BF16)
            w2sb = consts.tile([128, 9, 32], BF16)
            w3sb = consts.tile([128, G, 128], BF16)
            b1sb = consts.tile([128, 1], F32)
            b2sb = consts.tile([128, G], F32)
            b3sb = consts.tile([128, G], F32)
            idsb = consts.tile([128, 128], BF16)
            # lead both HWDGE rings with image 0's first half-superchunk
            # loads: conv1's first chunk-pair needs them, and the const
            # DMA issues otherwise delay them ~4us
            xg = {}
            for b in range(B):
                for g in range(G):
                    xg[(b, g)] = xpool.tile(
                        [128, PIX], BF16, name=f"x_{b}_{g}", tag="x"
                    )
            # image 0 arrives in chunk-pair order (all 4 groups' px quarter
            # q before quarter q+1) so conv1 cp0 can start ~4us in; conv1
            # needs every group's slice of the same pixels at once.
            QW = 2 * CH  # 784 px = one conv1 chunk-pair
            smt = {}
            sst = {}
            smt[0] = mpool.tile([128, 49], BF16, name="sm_0", tag="sm")
            sst[0] = mpool.tile([128, G * NSC * 49], BF16, name="ss_0", tag="ss")
            # all x quarters on sync: 16 issues x 0.6us matches the 9.5us
            # data time, so arrival is unchanged -- but the scalar engine
            # previously spent until ~17us ring-credit-blocked on its 8
            # quarter issues, head-of-line blocking the stage-A relus queued
            # behind them (first conv2 was 21.3us, not data-bound).
            # Consts + seeds (tiny) go on scalar, done by ~10us.
            nc.scalar.dma_start(out=w1sb, in_=w1l[:])
            nc.scalar.dma_start(out=b1sb, in_=b1d[:])
            nc.scalar.dma_start(out=smt[0], in_=smd[0])
            nc.scalar.dma_start(out=w2sb, in_=w2l[:])
            nc.scalar.dma_start(out=b2sb, in_=b2d[:])
            nc.scalar.dma_start(out=sst[0], in_=ssd[0])
            nc.scalar.dma_start(out=w3sb, in_=w3l[:])
            nc.scalar.dma_start(out=b3sb, in_=b3d[:])
            nc.scalar.dma_start(out=idsb, in_=idm[:])
            for q in range(PIX // QW):
                for g in range(G):
                    nc.sync.dma_start(
                        out=xg[(0, g)][:, QW * q : QW * (q + 1)],
                        in_=xs[0, 128 * g : 128 * (g + 1), QW * q : QW * (q + 1)],
                    )
            for b in range(1, B):
                # seeds first: mrow[b]'s DVE copy gets scheduled early and
                # head-of-line-blocks the stage-B muls if its seed is queued
                # behind 3.2MB of x loads
                smt[b] = mpool.tile([128, 49], BF16, name=f"sm_{b}", tag="sm")
                nc.scalar.dma_start(out=smt[b], in_=smd[b])
                sst[b] = mpool.tile(
                    [128, G * NSC * 49], BF16, name=f"ss_{b}", tag="ss"
                )
                nc.scalar.dma_start(out=sst[b], in_=ssd[b])
                for g in range(G):
                    nc.sync.dma_start(
                        out=xg[(b, g)], in_=xs[b, 128 * g : 128 * (g + 1), :]
                    )

            # PSUM bank plan (8 banks):
            #   pB0-3:   conv2, bank = group, held per superchunk (1 bank each)
            #   pC0/pC1: [128,1024] 2-bank tiles, double duty: conv1 output
            #            (alternating per chunk-pair) and conv3 (per half h).
            def pbank(name, tag, width=512):
                return psum.tile([128, 512], F32, name=name, tag=tag)[:, :width]

            def pbank2(name, tag):
                return psum.tile([128, 1024], F32, name=name, tag=tag)

            # PE warmup: keep TensorE busy during the input-DMA head so the
            # HAM clock gate reaches 8/8 before conv1; results are discarded.
            warm = pbank2("warm", "pC0")[:, :128]
            for wi in range(16):
                nc.tensor.matmul(
                    warm[0:32, 0:32],
                    w1sb[:, 0, :],
                    w1sb[:, 1, :],
                    start=True,
                    stop=True,
                    tile_position=(0, 0),
                )

            # stage C of superchunk s is emitted after conv2 of superchunk
            # s+1, so the dense conv2 matmul stream hides the a2 drain
            # latency that otherwise stalls the in-order PE queue.
            pending_c = []
            stc_idx = [0]
            half_ctr = [0]

            def emit_stage_c_unit(ctx, g):
                for h in range(2):
                    emit_stage_c_half(ctx, g, h)

            def emit_stage_c_half(ctx, g, h):
                b, k, a2s = ctx
                unit_idx = stc_idx[0] // 2
                path = STC_PATH[unit_idx % len(STC_PATH)]
                stc_idx[0] += 1
                if True:
                    p3 = pbank2(f"p3_{b}_{k}_{g}_{h}", f"pC{h}")
                    for dj in range(2):
                        j = 2 * h + dj
                        nc.tensor.matmul(
                            p3[:, 512 * dj : 512 * dj + CH],
                            w3sb[32 * j : 32 * (j + 1), g, :],
                            a2s[g][32 * j : 32 * (j + 1), :],
                            start=True,
                            stop=(path == "D"),
                            tile_position=(32 * j, 0),
                        )
                    if path == "P":
                        # fold the residual into psum with identity matmuls,
                        # then drain with one fused ACT relu+bias pass
                        for dj in range(2):
                            c = SC * k + 2 * h + dj
                            nc.tensor.matmul(
                                p3[:, 512 * dj : 512 * dj + CH],
                                idsb,
                                xg[(b, g)][:, CH * c : CH * (c + 1)],
                                start=False,
                                stop=True,
                                tile_position=(0, 0),
                            )
                    psv = p3.rearrange("q (c v) -> q c v", c=2)[:, :, :CH]
                    ot = opool.tile(
                        [128, 2 * CH], BF16, name=f"o_{b}_{k}_{g}_{h}", tag="o"
                    )
                    otv = ot.rearrange("q (c v) -> q c v", v=CH)
                    in_flush = unit_idx >= 12
                    use_act = (g < 2) and not in_flush
                    half_ctr[0] += 1
                    if path == "P":
                        nc.scalar.activation(
                            otv,
                            psv,
                            mybir.ActivationFunctionType.Relu,
                            bias=b3sb[:, g : g + 1],
                        )
                    elif use_act:
                        xgv = xg[(b, g)][
                            :, SCW * k + 2 * CH * h : SCW * k + 2 * CH * (h + 1)
                        ].rearrange("q (c v) -> q c v", v=CH)
                        tmp = upool.tile(
                            [128, 2 * CH], BF16,
                            name=f"t_{b}_{k}_{g}_{h}", tag="tmp",
                        )
                        nc.vector.scalar_tensor_tensor(
                            out=tmp.rearrange("q (c v) -> q c v", v=CH),
                            in0=psv,
                            scalar=b3sb[:, g : g + 1],
                            in1=xgv,
                            op0=mybir.AluOpType.add,
                            op1=mybir.AluOpType.add,
                        )
                        nc.scalar.activation(
                            otv,
                            tmp.rearrange("q (c v) -> q c v", v=CH),
                            mybir.ActivationFunctionType.Relu,
                        )
                    else:
                        xgv = xg[(b, g)][
                            :, SCW * k + 2 * CH * h : SCW * k + 2 * CH * (h + 1)
                        ].rearrange("q (c v) -> q c v", v=CH)
                        tmp = upool.tile(
                            [128, 2 * CH], BF16,
                            name=f"t_{b}_{k}_{g}_{h}", tag="tmp",
                        )
                        nc.vector.scalar_tensor_tensor(
                            out=tmp.rearrange("q (c v) -> q c v", v=CH),
                            in0=psv,
                            scalar=b3sb[:, g : g + 1],
                            in1=xgv,
                            op0=mybir.AluOpType.add,
                            op1=mybir.AluOpType.add,
                        )
                        # non-in-place max: in-place blocks the 4x bf16 mode
                        nc.vector.tensor_scalar_max(out=ot, in0=tmp, scalar1=0.0)
                    nc.sync.dma_start(
                        out=ys[
                            b,
                            128 * g : 128 * (g + 1),
                            SCW * k + 2 * CH * h : SCW * k + 2 * CH * (h + 1),
                        ],
                        in_=ot,
                    )

            for b in range(B):
                # ---- mask row-expansion: mrow[q, r, xc] = seed[q, r//8, xc]
                mrow = mpool.tile([128, H, 7], BF16, name=f"mrow_{b}", tag="mrow")
                nc.vector.tensor_copy(
                    out=mrow.rearrange("p (y dy) x -> p y dy x", dy=8),
                    in_=smt[b]
                    .rearrange("p (y x) -> p y x", x=7)
                    .unsqueeze(2)
                    .broadcast_to((128, 7, 8, 7)),
                )

                # ---- halo'd a1: zero only the border ring ----
                a1h = a1pool.tile([128, HH, HH], BF16, name=f"a1h_{b}", tag="a1h")
                nc.gpsimd.memset(a1h[:, 0, :], 0.0)
                nc.gpsimd.memset(a1h[:, HH - 1, :], 0.0)
                nc.vector.memset(a1h[:, 1 : HH - 1, 0], 0.0)
                nc.vector.memset(a1h[:, 1 : HH - 1, HH - 1], 0.0)

                # ---- stage A: conv1, two chunks per 2-bank psum slot;
                #      784-wide relu/bias (ACT) + broadcast mask-mul
                #      (gpsimd) -> a1h interior ----
                def emit_stage_a(cp):
                    p1 = pbank2(f"p1_{b}_{cp}", f"pC{cp % 2}")
                    for dc in range(2):
                        c = 2 * cp + dc
                        for g in range(G):
                            nc.tensor.matmul(
                                p1[32 * g : 32 * (g + 1), 512 * dc : 512 * dc + CH],
                                w1sb[:, g, :],
                                xg[(b, g)][:, CH * c : CH * (c + 1)],
                                start=True,
                                stop=True,
                                tile_position=(0, 32 * g),
                                skip_group_check=True,
                            )
                    u1 = upool.tile([128, 2 * CH], BF16, name=f"u1_{b}_{cp}", tag="u1")
                    nc.scalar.activation(
                        u1.rearrange("q (c v) -> q c v", v=CH),
                        p1.rearrange("q (c v) -> q c v", c=2)[:, :, :CH],
                        mybir.ActivationFunctionType.Relu,
                        bias=b1sb[:, 0:1],
                    )
                    c0 = 2 * cp
                    # image 0: alternate DVE/GP (both idle in the head);
                    # image 1: all DVE -- it is idle in the transition window
                    # (b0's stage C drained during b0k1) and DVE muls are
                    # 0.98us vs GP's 1.53us, shortening the a1h(b1) chain.
                    mul_eng = (
                        nc.vector
                        if (b > 0 or cp % 2 == 0)
                        else nc.gpsimd
                    )
                    mul_eng.tensor_mul(
                        a1h[:, 1 + R * c0 : 1 + R * (c0 + 2), 1 : 1 + W].rearrange(
                            "q a (x dx) -> q a x dx", dx=8
                        ),
                        u1.rearrange("q (a x dx) -> q a x dx", a=2 * R, dx=8),
                        mrow[:, R * c0 : R * (c0 + 2), :]
                        .unsqueeze(3)
                        .broadcast_to((128, 2 * R, 7, 8)),
                    )

                # conv2 k0's taps never read chunks 6-7 (cp3 feeds k1 only):
                # defer cp3 into k0's tap stream so conv2 starts ~3us earlier
                # (the PE queue is emission-ordered).
                for cp in range(NCH // 2 - 1):
                    emit_stage_a(cp)

                # ---- stage B per superchunk; the previous superchunk's
                # stage C units are interleaved into the tap stream so the
                # dense conv2 matmuls hide the drain latencies ----
                for k in range(NSC):
                    last_sc = b == B - 1 and k == NSC - 1

                    def emit_stage_b(g, p2g):
                        # a2 = m * relu(p2 + c2) = relu(m * (p2 + c2)).
                        # g0 runs fully on DVE (STT+max) for minimum latency:
                        # it gates the pending conv3(g0) at the next tap 1.
                        # Other groups: relu/bias (ACT) + mask-mul (GP/DVE).
                        at = a2pool.tile([128, CH], BF16, name=f"a2_{b}_{k}_{g}", tag="a2s")
                        off = (g * NSC + k) * 49
                        msl = (
                            sst[b][:, off : off + 49]
                            .rearrange("p (rl x) -> p rl x", x=7)
                            .unsqueeze(3)
                            .broadcast_to((128, R, 7, 8))
                        )
                        u2 = upool.tile([128, CH], BF16, name=f"u2_{b}_{k}_{g}", tag="u2")
                        nc.scalar.activation(
                            u2,
                            p2g,
                            mybir.ActivationFunctionType.Relu,
                            bias=b2sb[:, g : g + 1],
                        )
                        nc.gpsimd.tensor_mul(
                            at.rearrange("q (rl x dx) -> q rl x dx", rl=R, dx=8),
                            u2.rearrange("q (rl x dx) -> q rl x dx", rl=R, dx=8),
                            msl,
                        )
                        return at

                    p2 = [pbank(f"p2_{b}_{k}_{g}", f"pB{g}", CH) for g in range(G)]
                    a2s = {}
                    for t in range(9):
                        ky, kx = divmod(t, 3)
                        for g in range(G):
                            for j in range(SC):
                                c = SC * k + j
                                nc.tensor.matmul(
                                    p2[g][32 * j : 32 * (j + 1), :],
                                    w2sb[32 * g : 32 * (g + 1), t, :],
                                    a1h[
                                        32 * g : 32 * (g + 1),
                                        R * c + ky : R * c + ky + R,
                                        kx : kx + W,
                                    ],
                                    start=(t == 0),
                                    stop=(t == 8),
                                    tile_position=(32 * g, 32 * j),
                                    skip_group_check=True,
                                )
                        if t == 1 and k == 0:
                            emit_stage_a(NCH // 2 - 1)
                        if t >= 1 and pending_c:
                            emit_stage_c_half(pending_c[0], (t - 1) // 2, (t - 1) % 2)
                            if t == 8:
                                pending_c.pop(0)
                    for g in range(G):
                        a2s[g] = emit_stage_b(g, p2[g])
                    pending_c.append((b, k, a2s))

            while pending_c:
                ctx = pending_c.pop(0)
                for g in range(G):
                    emit_stage_c_unit(ctx, g)

    nc.finalize()
    return nc


def pack_params(w1, g1, b1, m1, v1, w2, g2, b2, m2, v2, w3, g3, b3, m3, v3):
    """Fold BN into weights/biases and lay out for the PE mappings."""
    import ml_dtypes

    f32 = np.float32
    bf16 = ml_dtypes.bfloat16
    s1 = (g1 / np.sqrt(v1 + EPS)).astype(f32)
    s2 = (g2 / np.sqrt(v2 + EPS)).astype(f32)
    s3 = (g3 / np.sqrt(v3 + EPS)).astype(f32)
    c1 = (b1 - m1 * s1).astype(f32)
    c2 = (b2 - m2 * s2).astype(f32)
    c3 = (b3 - m3 * s3).astype(f32)

    w1q = w1[:, :, 0, 0].astype(f32)  # [128 out, 128 in-per-group]
    w3q = w3[:, :, 0, 0].astype(f32)  # [512 out, 32 in-per-group]

    w1l = np.zeros([128, G, 32], f32)
    for g in range(G):
        blk = w1q[32 * g : 32 * (g + 1), :] * s1[32 * g : 32 * (g + 1), None]
        w1l[:, g, :] = blk.T  # [ci=128, co=32]

    w2l = np.zeros([128, 9, 32], f32)
    for g in range(G):
        sg = s2[32 * g : 32 * (g + 1), None]
        for t in range(9):
            ky, kx = divmod(t, 3)
            blk = w2[32 * g : 32 * (g + 1), :, ky, kx].astype(f32) * sg
            w2l[32 * g : 32 * (g + 1), t, :] = blk.T  # [ci=32, co=32]

    w3l = np.zeros([128, G, 128], f32)
    for g in range(G):
        blk = (w3q[128 * g : 128 * (g + 1), :] * s3[128 * g : 128 * (g + 1), None]).T
        for j in range(4):
            w3l[32 * j : 32 * (j + 1), g, :] = blk  # [ci=32, co=128], j-replicated

    b1v = c1.reshape(128, 1).astype(f32)
    b2v = np.zeros([128, G], f32)
    for g in range(G):
        for j in range(4):
            b2v[32 * j : 32 * (j + 1), g] = c2[32 * g : 32 * (g + 1)]
    b3v = c3.reshape(G, 128).T.astype(f32).copy()
    return dict(
        w1l=w1l.astype(bf16),
        w2l=w2l.astype(bf16),
        w3l=w3l.astype(bf16),
        b1d=b1v,
        b2d=b2v,
        b3d=b3v,
        idm=np.eye(128, dtype=f32).astype(bf16),
    )


def pack_seeds(mask):
    """[16, 4, 7, 7] -> bf16 mask seeds.

    smd[b, 32g+c, 7y+x]             = m[b, g, y, x]
    ssd[b, 32j+c, (g*NSC+k)*49 + 7*rl + xc] = m[b, g, (7*(4k+j)+rl)//8, xc]
    """
    import ml_dtypes

    bi = mask.shape[0]
    mf = mask.reshape(bi, G, 49)
    smd = np.repeat(mf, 32, axis=1)  # [b, 128, 49]
    ssd = np.zeros([bi, 128, G * NSC * 49], np.float32)
    for j in range(SC):
        for g in range(G):
            for k in range(NSC):
                c = SC * k + j
                for rl in range(R):
                    my = (R * c + rl) // 8
                    ssd[
                        :,
                        32 * j : 32 * (j + 1),
                        (g * NSC + k) * 49 + 7 * rl : (g * NSC + k) * 49 + 7 * rl + 7,
                    ] = mask[:, g, my, :][:, None, :]
    return (
        np.ascontiguousarray(smd).astype(ml_dtypes.bfloat16),
        np.ascontiguousarray(ssd).astype(ml_dtypes.bfloat16),
    )


def _run(inputs, **spmd_kwargs):
    import ml_dtypes

    x = np.asarray(inputs["x"], dtype=np.float32)
    mask = np.asarray(inputs["mask"], dtype=np.float32)
    params = pack_params(
        *(np.asarray(inputs[k], dtype=np.float32)
          for k in ("w1", "g1", "b1", "m1", "v1",
                    "w2", "g2", "b2", "m2", "v2",
                    "w3", "g3", "b3", "m3", "v3"))
    )
    smd, ssd = pack_seeds(mask)
    xr = np.ascontiguousarray(x.reshape(B_TOT, CIN, PIX)).astype(ml_dtypes.bfloat16)

    nc = build_nc()
    in_maps = []
    for c in range(N_CORES):
        sl = slice(B * c, B * (c + 1))
        m = {
            "xs": np.ascontiguousarray(xr[sl]),
            "smd": np.ascontiguousarray(smd[sl]),
            "ssd": np.ascontiguousarray(ssd[sl]),
        }
        m.update(params)
        in_maps.append(m)

    res = run_bass_kernel_spmd(nc, in_maps, core_ids=list(range(N_CORES)), **spmd_kwargs)
    out = np.concatenate([r["ys"] for r in res.results], axis=0)
    return out.astype(np.float32).reshape(B_TOT, CIN, H, W), res


def kernel(**inputs):
    out, _ = _run(inputs)
    return out


if __name__ == "__main__":
    # smoke: build only
    nc = build_nc()
    print("built ok")


# revision 45
# speedup vs baseline: 1.0188x; 1.0188x over previous
"""Trainium2 Bass kernel for nn_Bottleneck_refine (grouped bottleneck + block mask).

Reference computation (per image b):
    m   = upsample(mask[b])            # [4,7,7] -> per-group 56x56 {0,1}
    t1  = conv1x1_g4(x * m1)           # 512 -> 128; 1x1 commutes with mask
    a1  = m . relu(s1*t1 + c1)
    t2  = conv3x3_g4(a1)               # 128 -> 128 (pad 1)
    a2  = m . relu(s2*t2 + c2)
    y   = relu(s3*conv1x1_g4(a2) + c3 + x)

Identity used: for m in {0,1}:  m*relu(z) == relu(m*z), and the 1x1 conv
commutes with per-pixel masking, so the input mask multiply is absorbed.

v4-v8 (from v3, 87.0us; fast-mode HW exec ~79-82us, the device itself is
bimodal with a ~+13us slow clock mode):
  - masks ship as tiny seeds (12.5KB + 100KB per image vs 1.6MB) and are
    applied via 0-stride broadcast APs on DVE/GPSIMD; kills 3.2MB/core of
    HBM mask traffic plus the big mask SBUF tiles.
  - a1h is no longer fully memset: only the 58x58 halo border is zeroed
    (4 small strided memsets), saving ~5.5us of gpsimd.
  - per-image prep + all load dma_starts live in a prologue; image 0
    arrives in chunk-pair-ordered quarters split across both HWDGE rings
    (the head is issue-bound at ~0.6us/dma_start), everything else on the
    sync ring only -- a dma_start on a compute engine's queue blocks all
    later compute on that engine when the ring backs up.
  - stage C is one scalar_tensor_tensor (psum + c3 + x) followed by one
    relu, emitted per 784-px half at every conv2 tap of the next
    superchunk; relus alternate ACT (g<2, early taps) / DVE (late taps);
    the last four units split PE-identity-residual (P) and DVE-STT (D)
    paths so the tail drains on two engines in parallel.
  - v8 scheduler-aware fixes: image-1 seed loads issue BEFORE its x
    loads (mrow[b1]'s DVE copy is scheduled early and head-of-line
    blocked the stage-B muls for ~10us waiting on its seed); conv1 cp3
    is deferred into k0's tap stream (k0's taps never read chunks 6-7,
    and the PE queue is emission-ordered); all stage-B mask-muls on
    gpsimd (it idles between stage-A muls, and DVE was the 2nd-busiest).
  - measured engine facts: MM queue is serial (conv2 33ns/MM is the
    4-cols/beat psum write floor; 128-part-out MMs are 1 col/beat);
    gpsimd tensor_scalar_max is ~30x slow, gp tensor_mul is fine; DVE
    tensor_tensor-class ops run 1x from PSUM, tensor_scalar 2x.

Sharding: data-parallel over batch, 2 images per core on 8 cores.
Per-core HBM traffic ~= 6.7 MB in + 6.4 MB out (bf16).

Layouts per image (all SBUF, [partition, free]):
  xb_g    [128, 1568] bf16 per (group, superchunk) (channel-major, row-major px)
  a1h     [128, 58*58] bf16 halo'd masked mid activation
  a2s     [128, 392] bf16 per (g, superchunk): partition 32j+co = chunk 4k+j
  mrow    [128, 56, 7] bf16: mask value per (pixel row, mask col) per group
  chunks: 7 image rows (392 px), 8 chunks, 2 superchunks of 4 chunks.

PE mapping:
  conv1: 128x32 column tiling, tile (0, 32g), psum banks pC0/pC1 alternating.
  conv2: 32x32 16-tile packing, tile (32g, 32j): row=group, col=chunk-in-sc.
         9 taps accumulate into bank pB[g]; output chunk-scrambled.
  conv3: 32x128 row tiling, tile (32j, 0); 2 chunks per 2-bank psum tile
         (pC0/pC1 per half h).
"""

import numpy as np

import concourse.bass as bass
import concourse.tile as tile
from concourse import bacc, mybir
from concourse.bass_utils import run_bass_kernel_spmd

F32 = mybir.dt.float32
BF16 = mybir.dt.bfloat16
EPS = 1e-5

N_CORES = 8
B_TOT = 16
B = B_TOT // N_CORES  # images per core
G = 4
CIN = 512
MID = 128
H = W = 56
PIX = H * W  # 3136
R = 7  # image rows per chunk
CH = R * W  # 392 pixels per chunk
NCH = H // R  # 8 chunks
SC = 4  # chunks per superchunk
NSC = NCH // SC  # 2 superchunks
HH = H + 2  # halo'd height/width (58)
SCW = SC * CH  # pixels per superchunk (1568)

# stage C path per unit in emission order (16 units = (b,k) x g):
#   "D": one fused DVE STT (psum + c3 + x) -> tmp, then relu on ACT (g<2)
#        or a non-in-place DVE max (in-place blocks the 4x bf16 mode).
#        Keep the STT fused: splitting it into tensor_scalar+add to get the
#        2x-PSUM mode regressed ~18us twice -- per-op DVE DRAIN flushes make
#        op COUNT the currency, exactly as the engine doc warns.
#        (STT must stay off gpsimd: no PSUM access; gp max is ~13.5us!)
#   "P": residual folded into PSUM via an identity matmul (PE has slack in
#        the back half), drained by a single fused ACT relu+bias from psum.
STC_PATH = ["D"] * 12 + ["P", "D", "P", "D"]


def build_nc():
    # Bacc (not Bass): its compile()/finalize() pipeline legalizes sync waits
    # (>=2 waits per instruction are split into EventSemaphore instructions,
    # which this walrus build requires) and moves matmul waits to ldweights.
    nc = bacc.Bacc(None, target_bir_lowering=False)

    xs = nc.dram_tensor("xs", [B, CIN, PIX], BF16, kind="ExternalInput")
    smd = nc.dram_tensor("smd", [B, 128, 49], BF16, kind="ExternalInput")
    ssd = nc.dram_tensor("ssd", [B, 128, G * NSC * 49], BF16, kind="ExternalInput")
    w1l = nc.dram_tensor("w1l", [128, G, 32], BF16, kind="ExternalInput")
    w2l = nc.dram_tensor("w2l", [128, 9, 32], BF16, kind="ExternalInput")
    w3l = nc.dram_tensor("w3l", [128, G, 128], BF16, kind="ExternalInput")
    b1d = nc.dram_tensor("b1d", [128, 1], F32, kind="ExternalInput")
    b2d = nc.dram_tensor("b2d", [128, G], F32, kind="ExternalInput")
    b3d = nc.dram_tensor("b3d", [128, G], F32, kind="ExternalInput")
    idm = nc.dram_tensor("idm", [128, 128], BF16, kind="ExternalInput")
    ys = nc.dram_tensor("ys", [B, CIN, PIX], BF16, kind="ExternalOutput")

    with tile.TileContext(nc) as tc:
        with (
            tc.tile_pool(name="consts", bufs=1) as consts,
            tc.tile_pool(name="xpool", bufs=16) as xpool,
            tc.tile_pool(name="mpool", bufs=2) as mpool,
            tc.tile_pool(name="a1pool", bufs=2) as a1pool,
            tc.tile_pool(name="a2pool", bufs=12) as a2pool,
            tc.tile_pool(name="upool", bufs=14) as upool,
            tc.tile_pool(name="opool", bufs=12) as opool,
            tc.tile_pool(name="psum", bufs=1, space="PSUM") as psum,
        ):
            # ---- constants (loaded once) ----
            w1sb = consts.tile([128, G, 32], BF16)
            w2sb = consts.tile([128, 9, 32], BF16)
            w3sb = consts.tile([128, G, 128], BF16)
            b1sb = consts.tile([128, 1], F32)
            b2sb = consts.tile([128, G], F32)
            b3sb = consts.tile([128, G], F32)
            idsb = consts.tile([128, 128], BF16)
            # lead both HWDGE rings with image 0's first half-superchunk
            # loads: conv1's first chunk-pair needs them, and the const
            # DMA issues otherwise delay them ~4us
            xg = {}
            for b in range(B):
                for g in range(G):
                    xg[(b, g)] = xpool.tile(
                        [128, PIX], BF16, name=f"x_{b}_{g}", tag="x"
                    )
            # image 0 arrives in chunk-pair order (all 4 groups' px quarter
            # q before quarter q+1) so conv1 cp0 can start ~4us in; conv1
            # needs every group's slice of the same pixels at once.
            QW = 2 * CH  # 784 px = one conv1 chunk-pair
            smt = {}
            sst = {}
            smt[0] = mpool.tile([128, 49], BF16, name="sm_0", tag="sm")
            sst[0] = mpool.tile([128, G * NSC * 49], BF16, name="ss_0", tag="ss")
            # all x quarters on sync: 16 issues x 0.6us matches the 9.5us
            # data time, so arrival is unchanged -- but the scalar engine
            # previously spent until ~17us ring-credit-blocked on its 8
            # quarter issues, head-of-line blocking the stage-A relus queued
            # behind them (first conv2 was 21.3us, not data-bound).
            # Consts + seeds (tiny) go on scalar, done by ~10us.
            # the scalar engine issues ONLY the three loads stage A needs
            # first (done by ~8.8us) -- scalar-ring issues drain at 0.7-1.8us
            # each while sync hogs the SDMA engines, and every extra issue
            # head-of-line blocks the stage-A relus behind it (ACT was stuck
            # until 16.4us). Everything else rides sync between quarters.
            nc.scalar.dma_start(out=w1sb, in_=w1l[:])
            nc.scalar.dma_start(out=b1sb, in_=b1d[:])
            nc.scalar.dma_start(out=smt[0], in_=smd[0])
            for q in range(PIX // QW):
                for g in range(G):
                    nc.sync.dma_start(
                        out=xg[(0, g)][:, QW * q : QW * (q + 1)],
                        in_=xs[0, 128 * g : 128 * (g + 1), QW * q : QW * (q + 1)],
                    )
                if q == 0:
                    nc.sync.dma_start(out=b2sb, in_=b2d[:])
                    nc.sync.dma_start(out=sst[0], in_=ssd[0])
                elif q == 1:
                    nc.sync.dma_start(out=w2sb, in_=w2l[:])
                    nc.sync.dma_start(out=b3sb, in_=b3d[:])
                elif q == 2:
                    nc.sync.dma_start(out=w3sb, in_=w3l[:])
                    nc.sync.dma_start(out=idsb, in_=idm[:])
            for b in range(1, B):
                # seeds first: mrow[b]'s DVE copy gets scheduled early and
                # head-of-line-blocks the stage-B muls if its seed is queued
                # behind 3.2MB of x loads
                smt[b] = mpool.tile([128, 49], BF16, name=f"sm_{b}", tag="sm")
                nc.sync.dma_start(out=smt[b], in_=smd[b])
                sst[b] = mpool.tile(
                    [128, G * NSC * 49], BF16, name=f"ss_{b}", tag="ss"
                )
                nc.sync.dma_start(out=sst[b], in_=ssd[b])
                for g in range(G):
                    nc.sync.dma_start(
                        out=xg[(b, g)], in_=xs[b, 128 * g : 128 * (g + 1), :]
                    )

            # PSUM bank plan (8 banks):
            #   pB0-3:   conv2, bank = group, held per superchunk (1 bank each)
            #   pC0/pC1: [128,1024] 2-bank tiles, double duty: conv1 output
            #            (alternating per chunk-pair) and conv3 (per half h).
            def pbank(name, tag, width=512):
                return psum.tile([128, 512], F32, name=name, tag=tag)[:, :width]

            def pbank2(name, tag):
                return psum.tile([128, 1024], F32, name=name, tag=tag)

            # PE warmup: keep TensorE busy during the input-DMA head so the
            # HAM clock gate reaches 8/8 before conv1; results are discarded.
            warm = pbank2("warm", "pC0")[:, :128]
            for wi in range(16):
                nc.tensor.matmul(
                    warm[0:32, 0:32],
                    w1sb[:, 0, :],
                    w1sb[:, 1, :],
                    start=True,
                    stop=True,
                    tile_position=(0, 0),
                )

            # stage C of superchunk s is emitted after conv2 of superchunk
            # s+1, so the dense conv2 matmul stream hides the a2 drain
            # latency that otherwise stalls the in-order PE queue.
            pending_c = []
            stc_idx = [0]
            half_ctr = [0]

            def emit_stage_c_unit(ctx, g):
                for h in range(2):
                    emit_stage_c_half(ctx, g, h)

            def emit_stage_c_half(ctx, g, h):
                b, k, a2s = ctx
                unit_idx = stc_idx[0] // 2
                path = STC_PATH[unit_idx % len(STC_PATH)]
                stc_idx[0] += 1
                if True:
                    p3 = pbank2(f"p3_{b}_{k}_{g}_{h}", f"pC{h}")
                    for dj in range(2):
                        j = 2 * h + dj
                        nc.tensor.matmul(
                            p3[:, 512 * dj : 512 * dj + CH],
                            w3sb[32 * j : 32 * (j + 1), g, :],
                            a2s[g][32 * j : 32 * (j + 1), :],
                            start=True,
                            stop=(path == "D"),
                            tile_position=(32 * j, 0),
                        )
                    if path == "P":
                        # fold the residual into psum with identity matmuls,
                        # then drain with one fused ACT relu+bias pass
                        for dj in range(2):
                            c = SC * k + 2 * h + dj
                            nc.tensor.matmul(
                                p3[:, 512 * dj : 512 * dj + CH],
                                idsb,
                                xg[(b, g)][:, CH * c : CH * (c + 1)],
                                start=False,
                                stop=True,
                                tile_position=(0, 0),
                            )
                    psv = p3.rearrange("q (c v) -> q c v", c=2)[:, :, :CH]
                    ot = opool.tile(
                        [128, 2 * CH], BF16, name=f"o_{b}_{k}_{g}_{h}", tag="o"
                    )
                    otv = ot.rearrange("q (c v) -> q c v", v=CH)
                    in_flush = unit_idx >= 12
                    use_act = (g < 2) and not in_flush
                    half_ctr[0] += 1
                    if path == "P":
                        nc.scalar.activation(
                            otv,
                            psv,
                            mybir.ActivationFunctionType.Relu,
                            bias=b3sb[:, g : g + 1],
                        )
                    elif use_act:
                        xgv = xg[(b, g)][
                            :, SCW * k + 2 * CH * h : SCW * k + 2 * CH * (h + 1)
                        ].rearrange("q (c v) -> q c v", v=CH)
                        tmp = upool.tile(
                            [128, 2 * CH], BF16,
                            name=f"t_{b}_{k}_{g}_{h}", tag="tmp",
                        )
                        nc.vector.scalar_tensor_tensor(
                            out=tmp.rearrange("q (c v) -> q c v", v=CH),
                            in0=psv,
                            scalar=b3sb[:, g : g + 1],
                            in1=xgv,
                            op0=mybir.AluOpType.add,
                            op1=mybir.AluOpType.add,
                        )
                        nc.scalar.activation(
                            otv,
                            tmp.rearrange("q (c v) -> q c v", v=CH),
                            mybir.ActivationFunctionType.Relu,
                        )
                    else:
                        xgv = xg[(b, g)][
                            :, SCW * k + 2 * CH * h : SCW * k + 2 * CH * (h + 1)
                        ].rearrange("q (c v) -> q c v", v=CH)
                        tmp = upool.tile(
                            [128, 2 * CH], BF16,
                            name=f"t_{b}_{k}_{g}_{h}", tag="tmp",
                        )
                        nc.vector.scalar_tensor_tensor(
                            out=tmp.rearrange("q (c v) -> q c v", v=CH),
                            in0=psv,
                            scalar=b3sb[:, g : g + 1],
                            in1=xgv,
                            op0=mybir.AluOpType.add,
                            op1=mybir.AluOpType.add,
                        )
                        # non-in-place max: in-place blocks the 4x bf16 mode
                        nc.vector.tensor_scalar_max(out=ot, in0=tmp, scalar1=0.0)
                    nc.sync.dma_start(
                        out=ys[
                            b,
                            128 * g : 128 * (g + 1),
                            SCW * k + 2 * CH * h : SCW * k + 2 * CH * (h + 1),
                        ],
                        in_=ot,
                    )

            for b in range(B):
                # ---- mask row-expansion: mrow[q, r, xc] = seed[q, r//8, xc]
                mrow = mpool.tile([128, H, 7], BF16, name=f"mrow_{b}", tag="mrow")
                nc.vector.tensor_copy(
                    out=mrow.rearrange("p (y dy) x -> p y dy x", dy=8),
                    in_=smt[b]
                    .rearrange("p (y x) -> p y x", x=7)
                    .unsqueeze(2)
                    .broadcast_to((128, 7, 8, 7)),
                )

                # ---- halo'd a1: zero only the border ring ----
                a1h = a1pool.tile([128, HH, HH], BF16, name=f"a1h_{b}", tag="a1h")
                nc.gpsimd.memset(a1h[:, 0, :], 0.0)
                nc.gpsimd.memset(a1h[:, HH - 1, :], 0.0)
                nc.vector.memset(a1h[:, 1 : HH - 1, 0], 0.0)
                nc.vector.memset(a1h[:, 1 : HH - 1, HH - 1], 0.0)

                # ---- stage A: conv1, two chunks per 2-bank psum slot;
                #      784-wide relu/bias (ACT) + broadcast mask-mul
                #      (gpsimd) -> a1h interior ----
                def emit_stage_a(cp):
                    p1 = pbank2(f"p1_{b}_{cp}", f"pC{cp % 2}")
                    for dc in range(2):
                        c = 2 * cp + dc
                        for g in range(G):
                            nc.tensor.matmul(
                                p1[32 * g : 32 * (g + 1), 512 * dc : 512 * dc + CH],
                                w1sb[:, g, :],
                                xg[(b, g)][:, CH * c : CH * (c + 1)],
                                start=True,
                                stop=True,
                                tile_position=(0, 32 * g),
                                skip_group_check=True,
                            )
                    u1 = upool.tile([128, 2 * CH], BF16, name=f"u1_{b}_{cp}", tag="u1")
                    nc.scalar.activation(
                        u1.rearrange("q (c v) -> q c v", v=CH),
                        p1.rearrange("q (c v) -> q c v", c=2)[:, :, :CH],
                        mybir.ActivationFunctionType.Relu,
                        bias=b1sb[:, 0:1],
                    )
                    c0 = 2 * cp
                    # image 0: alternate DVE/GP (both idle in the head);
                    # image 1: all DVE -- it is idle in the transition window
                    # (b0's stage C drained during b0k1) and DVE muls are
                    # 0.98us vs GP's 1.53us, shortening the a1h(b1) chain.
                    mul_eng = (
                        nc.vector
                        if (b > 0 or cp % 2 == 0)
                        else nc.gpsimd
                    )
                    mul_eng.tensor_mul(
                        a1h[:, 1 + R * c0 : 1 + R * (c0 + 2), 1 : 1 + W].rearrange(
                            "q a (x dx) -> q a x dx", dx=8
                        ),
                        u1.rearrange("q (a x dx) -> q a x dx", a=2 * R, dx=8),
                        mrow[:, R * c0 : R * (c0 + 2), :]
                        .unsqueeze(3)
                        .broadcast_to((128, 2 * R, 7, 8)),
                    )

                # conv2 k0's taps never read chunks 6-7 (cp3 feeds k1 only):
                # defer cp3 into k0's tap stream so conv2 starts ~3us earlier
                # (the PE queue is emission-ordered).
                for cp in range(NCH // 2 - 1):
                    emit_stage_a(cp)

                # ---- stage B per superchunk; the previous superchunk's
                # stage C units are interleaved into the tap stream so the
                # dense conv2 matmuls hide the drain latencies ----
                for k in range(NSC):
                    last_sc = b == B - 1 and k == NSC - 1

                    def emit_stage_b(g, p2g):
                        # a2 = m * relu(p2 + c2) = relu(m * (p2 + c2)).
                        # g0 runs fully on DVE (STT+max) for minimum latency:
                        # it gates the pending conv3(g0) at the next tap 1.
                        # Other groups: relu/bias (ACT) + mask-mul (GP/DVE).
                        at = a2pool.tile([128, CH], BF16, name=f"a2_{b}_{k}_{g}", tag="a2s")
                        off = (g * NSC + k) * 49
                        msl = (
                            sst[b][:, off : off + 49]
                            .rearrange("p (rl x) -> p rl x", x=7)
                            .unsqueeze(3)
                            .broadcast_to((128, R, 7, 8))
                        )
                        u2 = upool.tile([128, CH], BF16, name=f"u2_{b}_{k}_{g}", tag="u2")
                        nc.scalar.activation(
                            u2,
                            p2g,
                            mybir.ActivationFunctionType.Relu,
                            bias=b2sb[:, g : g + 1],
                        )
                        nc.gpsimd.tensor_mul(
                            at.rearrange("q (rl x dx) -> q rl x dx", rl=R, dx=8),
                            u2.rearrange("q (rl x dx) -> q rl x dx", rl=R, dx=8),
                            msl,
                        )
                        return at

                    p2 = [pbank(f"p2_{b}_{k}_{g}", f"pB{g}", CH) for g in range(G)]
                    a2s = {}
                    for t in range(9):
                        ky, kx = divmod(t, 3)
                        for g in range(G):
                            for j in range(SC):
                                c = SC * k + j
                                nc.tensor.matmul(
                                    p2[g][32 * j : 32 * (j + 1), :],
                                    w2sb[32 * g : 32 * (g + 1), t, :],
                                    a1h[
                                        32 * g : 32 * (g + 1),
                                        R * c + ky : R * c + ky + R,
                                        kx : kx + W,
                                    ],
                                    start=(t == 0),
                                    stop=(t == 8),
                                    tile_position=(32 * g, 32 * j),
                                    skip_group_check=True,
                                )
                        if t == 1 and k == 0:
                            emit_stage_a(NCH // 2 - 1)
                        if t >= 1 and pending_c:
                            emit_stage_c_half(pending_c[0], (t - 1) // 2, (t - 1) % 2)
                            if t == 8:
                                pending_c.pop(0)
                    for g in range(G):
                        a2s[g] = emit_stage_b(g, p2[g])
                    pending_c.append((b, k, a2s))

            while pending_c:
                ctx = pending_c.pop(0)
                for g in range(G):
                    emit_stage_c_unit(ctx, g)

    nc.finalize()
    return nc


def pack_params(w1, g1, b1, m1, v1, w2, g2, b2, m2, v2, w3, g3, b3, m3, v3):
    """Fold BN into weights/biases and lay out for the PE mappings."""
    import ml_dtypes

    f32 = np.float32
    bf16 = ml_dtypes.bfloat16
    s1 = (g1 / np.sqrt(v1 + EPS)).astype(f32)
    s2 = (g2 / np.sqrt(v2 + EPS)).astype(f32)
    s3 = (g3 / np.sqrt(v3 + EPS)).astype(f32)
    c1 = (b1 - m1 * s1).astype(f32)
    c2 = (b2 - m2 * s2).astype(f32)
    c3 = (b3 - m3 * s3).astype(f32)

    w1q = w1[:, :, 0, 0].astype(f32)  # [128 out, 128 in-per-group]
    w3q = w3[:, :, 0, 0].astype(f32)  # [512 out, 32 in-per-group]

    w1l = np.zeros([128, G, 32], f32)
    for g in range(G):
        blk = w1q[32 * g : 32 * (g + 1), :] * s1[32 * g : 32 * (g + 1), None]
        w1l[:, g, :] = blk.T  # [ci=128, co=32]

    w2l = np.zeros([128, 9, 32], f32)
    for g in range(G):
        sg = s2[32 * g : 32 * (g + 1), None]
        for t in range(9):
            ky, kx = divmod(t, 3)
            blk = w2[32 * g : 32 * (g + 1), :, ky, kx].astype(f32) * sg
            w2l[32 * g : 32 * (g + 1), t, :] = blk.T  # [ci=32, co=32]

    w3l = np.zeros([128, G, 128], f32)
    for g in range(G):
        blk = (w3q[128 * g : 128 * (g + 1), :] * s3[128 * g : 128 * (g + 1), None]).T
        for j in range(4):
            w3l[32 * j : 32 * (j + 1), g, :] = blk  # [ci=32, co=128], j-replicated

    b1v = c1.reshape(128, 1).astype(f32)
    b2v = np.zeros([128, G], f32)
    for g in range(G):
        for j in range(4):
            b2v[32 * j : 32 * (j + 1), g] = c2[32 * g : 32 * (g + 1)]
    b3v = c3.reshape(G, 128).T.astype(f32).copy()
    return dict(
        w1l=w1l.astype(bf16),
        w2l=w2l.astype(bf16),
        w3l=w3l.astype(bf16),
        b1d=b1v,
        b2d=b2v,
        b3d=b3v,
        idm=np.eye(128, dtype=f32).astype(bf16),
    )


def pack_seeds(mask):
    """[16, 4, 7, 7] -> bf16 mask seeds.

    smd[b, 32g+c, 7y+x]             = m[b, g, y, x]
    ssd[b, 32j+c, (g*NSC+k)*49 + 7*rl + xc] = m[b, g, (7*(4k+j)+rl)//8, xc]
    """
    import ml_dtypes

    bi = mask.shape[0]
    mf = mask.reshape(bi, G, 49)
    smd = np.repeat(mf, 32, axis=1)  # [b, 128, 49]
    ssd = np.zeros([bi, 128, G * NSC * 49], np.float32)
    for j in range(SC):
        for g in range(G):
            for k in range(NSC):
                c = SC * k + j
                for rl in range(R):
                    my = (R * c + rl) // 8
                    ssd[
                        :,
                        32 * j : 32 * (j + 1),
                        (g * NSC + k) * 49 + 7 * rl : (g * NSC + k) * 49 + 7 * rl + 7,
                    ] = mask[:, g, my, :][:, None, :]
    return (
        np.ascontiguousarray(smd).astype(ml_dtypes.bfloat16),
        np.ascontiguousarray(ssd).astype(ml_dtypes.bfloat16),
    )


def _run(inputs, **spmd_kwargs):
    import ml_dtypes

    x = np.asarray(inputs["x"], dtype=np.float32)
    mask = np.asarray(inputs["mask"], dtype=np.float32)
    params = pack_params(
        *(np.asarray(inputs[k], dtype=np.float32)
          for k in ("w1", "g1", "b1", "m1", "v1",
                    "w2", "g2", "b2", "m2", "v2",
                    "w3", "g3", "b3", "m3", "v3"))
    )
    smd, ssd = pack_seeds(mask)
    xr = np.ascontiguousarray(x.reshape(B_TOT, CIN, PIX)).astype(ml_dtypes.bfloat16)

    nc = build_nc()
    in_maps = []
    for c in range(N_CORES):
        sl = slice(B * c, B * (c + 1))
        m = {
            "xs": np.ascontiguousarray(xr[sl]),
            "smd": np.ascontiguousarray(smd[sl]),
            "ssd": np.ascontiguousarray(ssd[sl]),
        }
        m.update(params)
        in_maps.append(m)

    res = run_bass_kernel_spmd(nc, in_maps, core_ids=list(range(N_CORES)), **spmd_kwargs)
    out = np.concatenate([r["ys"] for r in res.results], axis=0)
    return out.astype(np.float32).reshape(B_TOT, CIN, H, W), res


def kernel(**inputs):
    out, _ = _run(inputs)
    return out


if __name__ == "__main__":
    # smoke: build only
    nc = build_nc()
    print("built ok")
